# revision 10
# baseline (speedup 1.0000x reference)
"""Trainium2 Bass kernel for MergedColumnParallelLinearWithLoRA.

Computes  out = x @ W.T + concat(lora1(x), lora2(x))  where
lora_i(x)[t] = B_i[l_t] @ (A_i[l_t] @ x[t]) + bias_i[l_t],  l_t = indices[t].

Sharding: ROW-parallel (token-sharded) across 8 NeuronCores, with tokens
globally SORTED by lora id on the host. Core c owns 1024 consecutive sorted
tokens; each 128-token tile then spans a tiny contiguous lora window
(WLOR loras; span is 1-2 for uniform routing). W is streamed in full by
every core; no collectives. Host un-permutes the output rows.

Per-core device program (all matmul operands bf16, fp32 PSUM accumulate):
  - x^T resident in SBUF ([128, 8, 16, 128] d-major tiles), bf16.
  - Phase A (shrink): per tile/slice, s^T = A_window @ x_t^T computed directly
    in transposed form (A-window stationary, x^T moving). Masked dispatch +
    bias indicators come from a host-built 0/1 mask tile:
      st[coords] = s^T * mask   (coords of lora wg+j zeroed unless idx==wg+j)
      st[ind]    = mask rows    ((idx == wg+j); multiplies the bias row)
  - Phase B: 22 output chunks of 512 cols. Per (chunk, tile): 16 streamed
    base matmuls (x^T stationary, W moving) + ONE fused LoRA-expand+bias
    matmul (st stationary, [B_rows; bias_rows] window moving) accumulating
    into the same PSUM bank (start=False). No bias gather, no one-hot.
  - WLOR=2 fast path: tile PAIRS are packed on partitions (even tile at rows
    0:34, odd at 64:98). The two shrink matmuls of a pair are col-tiled
    (out partitions 0:32 / 64:96) and the two expand matmuls are row-tiled
    (K rows 0:34 / 64:98), each issued adjacently -> they run CONCURRENTLY
    on disjoint 32x32 sub-array groups of the PE. WLOR=4 fallback (any
    routing with per-tile span <= 4) is unpacked.
  - PSUM -> SBUF copies split across ScalarE/VectorE (fp32 -> bf16), output
    DMA'd in bf16; host upcasts.

Rel error ~2.9e-3 (bf16 operands + bf16 output rounding) vs the 2e-2 gate.
"""

import numpy as np
import ml_dtypes

import concourse.bass as bass  # noqa: F401
import concourse.mybir as mybir
import concourse.tile as tile
from concourse import bacc

T, D, O, L, R = 8192, 2048, 5632, 16, 16
NCORES = 8
TL = T // NCORES  # 1024 tokens per core
P = 128
KT = D // P  # 16 k-tiles
MTL = TL // P  # 8 local token tiles (4 pairs)
NPAIR = MTL // 2
NF = 2 * O  # 11264 output columns
NCH = NF // 512  # 22 chunks
NCS = NCH // 2  # 11 per slice
ODD = 64  # partition offset of the odd tile in a packed pair

F32 = mybir.dt.float32
BF16 = mybir.dt.bfloat16
BF = ml_dtypes.bfloat16


def build_nc(reps=1, wlor=2):
    """wlor=2: packed tile-pair fast path; wlor=4: unpacked fallback."""
    sc = wlor * R
    sb = sc + wlor
    packed = wlor == 2
    nc = bacc.Bacc("TRN2", target_bir_lowering=False, debug=False)

    xt = nc.dram_tensor("xt", [MTL, P, KT, P], BF16, kind="ExternalInput")
    wt = nc.dram_tensor("wt", [NCH, P, KT, 512], BF16, kind="ExternalInput")
    aw = nc.dram_tensor("aw", [P, MTL, 2, KT, sc], BF16, kind="ExternalInput")
    if packed:
        bw = nc.dram_tensor("bw", [2, P, NPAIR, O], BF16, kind="ExternalInput")
        mm = nc.dram_tensor("mm", [P, NPAIR, P], BF16, kind="ExternalInput")
    else:
        bw = nc.dram_tensor("bw", [2, sb, MTL, O], BF16, kind="ExternalInput")
        mm = nc.dram_tensor("mm", [sb, MTL, P], BF16, kind="ExternalInput")
    out = nc.dram_tensor("out", [TL, NF], BF16, kind="ExternalOutput")

    with tile.TileContext(nc) as tc:
        with (
            tc.tile_pool(name="const", bufs=1) as const,
            tc.tile_pool(name="awpool", bufs=2) as awpool,
            tc.tile_pool(name="stpool", bufs=2) as stpool,
            tc.tile_pool(name="wpool", bufs=2) as wpool,
            tc.tile_pool(name="bwpool", bufs=2) as bwpool,
            tc.tile_pool(name="opool", bufs=4) as opool,
            tc.tile_pool(name="ps", bufs=8, space="PSUM") as ps,
        ):
            # ---------------- resident constants ----------------
            t_xr = const.tile([P, MTL, KT, P], BF16, tag="xr", name="t_xr")
            for kk in range(KT):
                nc.sync.dma_start(t_xr[:, 0, kk], xt[0, :, kk])
            for mtl in range(1, MTL):
                nc.sync.dma_start(t_xr[:, mtl], xt[mtl])
            t_mm = const.tile(
                [P, NPAIR, P] if packed else [sb, MTL, P],
                BF16, tag="mm", name="t_mm",
            )
            nc.sync.dma_start(t_mm[:], mm[:])

            for _rep in range(reps):
                # ---------------- phase A: LoRA shrink ----------------
                t_aw = awpool.tile([P, MTL, 2, KT, sc], BF16, tag="aw", name="t_aw")
                for mtl in range(MTL):
                    nc.sync.dma_start(t_aw[:, mtl], aw[:, mtl])
                if packed:
                    t_st = stpool.tile([P, 2, NPAIR, P], BF16, tag="st", name="t_st")
                    for pr in range(NPAIR):
                        te, to = 2 * pr, 2 * pr + 1
                        p_s = ps.tile([ODD + sc, 2, P], F32, tag="b", name="p_s")
                        for s in range(2):
                            for kk in range(KT):
                                nc.tensor.matmul(
                                    p_s[0:sc, s, :],
                                    t_aw[:, te, s, kk, :],
                                    t_xr[:, te, kk, :],
                                    start=(kk == 0),
                                    stop=(kk == KT - 1),
                                    skip_group_check=True,
                                )
                                nc.tensor.matmul(
                                    p_s[ODD : ODD + sc, s, :],
                                    t_aw[:, to, s, kk, :],
                                    t_xr[:, to, kk, :],
                                    start=(kk == 0),
                                    stop=(kk == KT - 1),
                                    skip_group_check=True,
                                )
                        for s in range(2):
                            for off in (0, ODD):
                                nc.vector.tensor_tensor(
                                    t_st[off : off + sc, s, pr, :],
                                    p_s[off : off + sc, s, :],
                                    t_mm[off : off + sc, pr, :],
                                    op=mybir.AluOpType.mult,
                                )
                                nc.scalar.copy(
                                    t_st[off + sc : off + sb, s, pr, :],
                                    t_mm[off + sc : off + sb, pr, :],
                                )
                else:
                    t_st = stpool.tile([sb, 2, MTL, P], BF16, tag="st", name="t_st")
                    for mtl in range(MTL):
                        for s in range(2):
                            p_s = ps.tile([sc, P], F32, tag="b", name="p_s")
                            for kk in range(KT):
                                nc.tensor.matmul(
                                    p_s[:],
                                    t_aw[:, mtl, s, kk, :],
                                    t_xr[:, mtl, kk, :],
                                    start=(kk == 0),
                                    stop=(kk == KT - 1),
                                )
                            nc.vector.tensor_tensor(
                                t_st[0:sc, s, mtl, :],
                                p_s[:],
                                t_mm[0:sc, mtl, :],
                                op=mybir.AluOpType.mult,
                            )
                            nc.scalar.copy(
                                t_st[sc:sb, s, mtl, :], t_mm[sc:sb, mtl, :]
                            )

                # ---------------- phase B: base + fused expand ----------------
                for ch in range(NCH):
                    s, ci = divmod(ch, NCS)
                    t_wc = wpool.tile([P, KT, 512], BF16, tag="w", name="t_wc")
                    for kk in range(KT):
                        nc.sync.dma_start(t_wc[:, kk], wt[ch, :, kk])
                    t_bw = bwpool.tile(
                        [P, NPAIR, 512] if packed else [sb, MTL, 512],
                        BF16, tag="bw", name="t_bw",
                    )
                    nc.sync.dma_start(
                        t_bw[:], bw[s, :, :, ci * 512 : (ci + 1) * 512]
                    )
                    if packed:
                        for pr in range(NPAIR):
                            te, to = 2 * pr, 2 * pr + 1
                            p_e = ps.tile([P, 512], F32, tag="b", name="p_e")
                            p_o = ps.tile([P, 512], F32, tag="b", name="p_o")
                            for kk in range(KT):
                                nc.tensor.matmul(
                                    p_e[:],
                                    t_xr[:, te, kk, :],
                                    t_wc[:, kk, :],
                                    start=(kk == 0),
                                    stop=False,
                                )
                            for kk in range(KT):
                                nc.tensor.matmul(
                                    p_o[:],
                                    t_xr[:, to, kk, :],
                                    t_wc[:, kk, :],
                                    start=(kk == 0),
                                    stop=False,
                                )
                            nc.tensor.matmul(
                                p_e[:],
                                t_st[0:sb, s, pr, :],
                                t_bw[0:sb, pr, :],
                                start=False,
                                stop=True,
                            )
                            nc.tensor.matmul(
                                p_o[:],
                                t_st[ODD : ODD + sb, s, pr, :],
                                t_bw[ODD : ODD + sb, pr, :],
                                start=False,
                                stop=True,
                            )
                            for half, p_x in ((0, p_e), (1, p_o)):
                                t_out = opool.tile(
                                    [P, 512], BF16, tag="o", name="t_out"
                                )
                                if (ch + pr + half) % 2 == 0:
                                    nc.vector.tensor_copy(t_out[:], p_x[:])
                                else:
                                    nc.scalar.copy(t_out[:], p_x[:])
                                mtl = 2 * pr + half
                                # out goes on the ACT HWDGE ring so it can
                                # never head-of-line-block wt/bw prefetches
                                # on the sync ring
                                nc.scalar.dma_start(
                                    out[
                                        mtl * P : (mtl + 1) * P,
                                        ch * 512 : (ch + 1) * 512,
                                    ],
                                    t_out[:],
                                )
                    else:
                        for mtl in range(MTL):
                            p_b = ps.tile([P, 512], F32, tag="b", name="p_b")
                            for kk in range(KT):
                                nc.tensor.matmul(
                                    p_b[:],
                                    t_xr[:, mtl, kk, :],
                                    t_wc[:, kk, :],
                                    start=(kk == 0),
                                    stop=False,
                                )
                            nc.tensor.matmul(
                                p_b[:],
                                t_st[:, s, mtl, :],
                                t_bw[:, mtl, :],
                                start=False,
                                stop=True,
                            )
                            t_out = opool.tile([P, 512], BF16, tag="o", name="t_out")
                            if (ch + mtl) % 2 == 0:
                                nc.vector.tensor_copy(t_out[:], p_b[:])
                            else:
                                nc.scalar.copy(t_out[:], p_b[:])
                            nc.sync.dma_start(
                                out[
                                    mtl * P : (mtl + 1) * P,
                                    ch * 512 : (ch + 1) * 512,
                                ],
                                t_out[:],
                            )

    nc.compile()
    return nc


# ---------------------------------------------------------------------------
# host-side sharding / unsharding
# ---------------------------------------------------------------------------


def _prep(x, W, lora_a1, lora_a2, lora_b1, lora_b2, bias1, bias2, indices,
          wlor=None):
    x = np.asarray(x, np.float32)
    W = np.asarray(W, np.float32)
    indices = np.asarray(indices, np.int32)

    perm = np.argsort(indices, kind="stable")
    idx_s = indices[perm]
    x_s = x[perm]

    # worst per-tile contiguous lora span decides the window size
    tiles = idx_s.reshape(NCORES * MTL, P)
    span = int((tiles.max(axis=1) - tiles.min(axis=1) + 1).max())
    if wlor is None:
        wlor = 2 if span <= 2 else 4
    if span > wlor:
        raise ValueError(f"tile lora span {span} exceeds window {wlor}")
    sc = wlor * R
    sb = sc + wlor
    packed = wlor == 2

    wt = np.ascontiguousarray(
        W.T.reshape(KT, P, NCH, 512).transpose(2, 1, 0, 3)
    ).astype(BF)

    A = [
        np.asarray(lora_a1, np.float32).transpose(2, 0, 1),  # [d, lora, r]
        np.asarray(lora_a2, np.float32).transpose(2, 0, 1),
    ]
    B = [np.asarray(lora_b1, np.float32), np.asarray(lora_b2, np.float32)]
    bias = [np.asarray(bias1, np.float32), np.asarray(bias2, np.float32)]

    xts = x_s.reshape(NCORES, MTL, P, KT, P).transpose(0, 1, 4, 3, 2).astype(BF)

    lid = np.concatenate([np.arange(wlor).repeat(R), np.arange(wlor)])  # [sb]

    in_maps = []
    for c in range(NCORES):
        idx_c = idx_s[c * TL : (c + 1) * TL].reshape(MTL, P)
        wg = np.clip(idx_c.min(axis=1), 0, L - wlor)

        awc = np.empty((P, MTL, 2, KT, sc), np.float32)
        if packed:
            bwc = np.zeros((2, P, NPAIR, O), np.float32)
            mmc = np.zeros((P, NPAIR, P), np.float32)
        else:
            bwc = np.zeros((2, sb, MTL, O), np.float32)
            mmc = np.zeros((sb, MTL, P), np.float32)
        for t in range(MTL):
            w0 = int(wg[t])
            if packed:
                pr, half = divmod(t, 2)
                off = ODD * half
            else:
                pr, off = t, 0
            for s in range(2):
                Awin = A[s][:, w0 : w0 + wlor, :].reshape(D, sc)
                awc[:, t, s] = Awin.reshape(KT, P, sc).transpose(1, 0, 2)
                bwc[s, off : off + sc, pr] = (
                    B[s][w0 : w0 + wlor].transpose(0, 2, 1).reshape(sc, O)
                )
                bwc[s, off + sc : off + sb, pr] = bias[s][w0 : w0 + wlor]
            mmc[off : off + sb, pr, :] = (
                idx_c[t][None, :] == (w0 + lid)[:, None]
            ).astype(np.float32)

        in_maps.append(
            {
                "xt": np.ascontiguousarray(xts[c]),
                "wt": wt,
                "aw": np.ascontiguousarray(awc.astype(BF)),
                "bw": np.ascontiguousarray(bwc.astype(BF)),
                "mm": np.ascontiguousarray(mmc.astype(BF)),
            }
        )
    return in_maps, perm, wlor


def shard_inputs(**inputs):
    return _prep(**inputs)[0]


def unshard_output(results, perm):
    out = np.empty((T, NF), np.float32)
    sorted_out = np.concatenate(
        [np.asarray(results[c]["out"], dtype=BF) for c in range(NCORES)], axis=0
    ).astype(np.float32)
    out[perm] = sorted_out
    return out


_CACHE = {}


def get_nc(wlor=2):
    key = f"nc{wlor}"
    if key not in _CACHE:
        _CACHE[key] = build_nc(wlor=wlor)
    return _CACHE[key]


def kernel(**inputs):
    from concourse import bass2jax

    in_maps, perm, wlor = _prep(**inputs)
    nc = get_nc(wlor)
    results = bass2jax.run_bass_via_pjrt(nc, in_maps, n_cores=NCORES)
    return unshard_output(results, perm)
